# revision 36
# baseline (speedup 1.0000x reference)
"""Trainium2 Bass kernel for DeepseekV4 HCA compressor (single-shot window compression).

Computation per 128-token window:
    kv   = h @ w_kv            [128, 128]
    gate = h @ w_gate + bias   [128, 128]
    w    = softmax(gate, axis=tokens)   (per output channel)
    comp = sum(w * kv, axis=tokens)     [128]
then RMS-norm over channels and interleaved RoPE on the last 64 channels.

Sharding: 128 windows (2 batches x 64) split across 8 cores, 16 windows each.

Layout strategy: h is uploaded PRE-TRANSPOSED per core ([H, tokens]) in bf16,
so the PE does zero transposes -- just the two bf16 projection matmuls at
full PE rate plus a tiny identity matmul folding in the position bias.

DMA transfers serialize on one shared engine (~360 GB/s/core), so the
whole input rides ONE SP-queue stream in exact consumption order:
[bias|wg chunks interleaved with h group-0 pieces, wkv halves], then 4kc
h pieces per group (cswn mid-stream). The PE is the critical engine
(~29us of bf16 matmuls vs ~27us of wire), so no dummy warmup: the clock
ramp is burned on the four bias matmuls + the group-0 DMA chase.

Per group: bias matmul opens the gate PSUM group (all four emitted first),
gate/kv matmuls interleave per 4-chunk quartet with the gate group closing
two quartets early -- the exp (ACT) and denominator reduce+recip (DVE)
finish inside the kv tail, leaving only e*kv + numerator reduce + num/den
after the last matmul. Epilogue is fused and runs once: one PE transpose
[128,16]->[16,128], ACT Square with fused row-sum straight off PSUM, DVE
1-Newton fast-rsqrt, scale (wn folded into the host cs tables for the
rope half), RoPE, and a two-queue output DMA (nope half on SP, rope half
on ACT) so the descriptor generations overlap.
"""

import sys

if "/opt/trn_rl_repo" not in sys.path:
    sys.path.insert(0, "/opt/trn_rl_repo")

import ml_dtypes
import numpy as np

import concourse.bacc as bacc
import concourse.mybir as mybir
import concourse.tile as tile
from concourse.bass_utils import run_bass_kernel_spmd
from concourse.masks import make_identity

# Problem shapes (hardcoded per contest contract)
B, S, H = 2, 8192, 2048
M = 128          # compress rate (window length)
D = 128          # head dim
T = S // M       # 64 windows per batch
NCORES = 8
WPC = (B * T) // NCORES   # 16 windows per core
GW = 4                    # windows per group (-> moving dim 512)
GROUPS = WPC // GW        # 4
KC = H // 128             # 16 contraction chunks
GM = GW * M               # 512 moving tokens per group
ROPE_DIM = 64
HALF = ROPE_DIM // 2
THETA = 10000.0
EPS = 1e-6

F32 = mybir.dt.float32
BF16 = mybir.dt.bfloat16
I32 = mybir.dt.int32
AF = mybir.ActivationFunctionType
ALU = mybir.AluOpType

NP_BF16 = ml_dtypes.bfloat16

# No PE warmup: the PE is the critical engine (slower per h-chunk than the
# DMA stream), so burning PE time on dummies to buy clock ramp is a net
# loss -- the ramp period overlaps the group-0 DMA chase instead.


def _build_nc():
    nc = bacc.Bacc(None, target_bir_lowering=False)

    # h for this core, transposed on host: hT[h, t] = h[t, h], bf16
    hT_in = nc.dram_tensor("hT_in", [H, WPC * M], BF16, kind="ExternalInput")
    # w_gate pre-permuted to [p, kc, d] plus the 4x-tiled position bias
    wgb_in = nc.dram_tensor("wgb_in", [128, KC * D + GM], BF16, kind="ExternalInput")
    wkv_in = nc.dram_tensor("wkv_in", [128, KC * D], BF16, kind="ExternalInput")
    # cos/sin table (128 cols) + rms weight row-broadcast (128 cols)
    cswn_in = nc.dram_tensor("cswn_in", [128, 2 * ROPE_DIM + D], F32, kind="ExternalInput")
    out_d = nc.dram_tensor("out_d", [WPC, D], F32, kind="ExternalOutput")

    with tile.TileContext(nc) as tc:
        with (
            tc.tile_pool(name="constp", bufs=1) as constp,
            tc.tile_pool(name="hTp", bufs=4) as hTp,
            tc.tile_pool(name="esbp", bufs=2) as esbp,
            tc.tile_pool(name="smallp", bufs=2) as smallp,
            tc.tile_pool(name="gtp", bufs=4, space="PSUM") as gtp,
            tc.tile_pool(name="kvp", bufs=3, space="PSUM") as kvp,
            tc.tile_pool(name="ctp", bufs=1, space="PSUM") as ctp,
            tc.tile_pool(name="finalp", bufs=1) as finalp,
        ):
            # --- constants / epilogue state ---
            ident = constp.tile([128, 128], F32, name="ident")
            make_identity(nc, ident)
            ident_bf = constp.tile([128, 128], BF16, name="ident_bf")
            nc.vector.tensor_copy(ident_bf[:, :], ident[:, :])

            comp = constp.tile([D, WPC], F32, name="comp")
            out_sb = finalp.tile([128, D], F32, name="out_sb")
            ssq = finalp.tile([128, 1], F32, name="ssq")
            sqs = finalp.tile([128, D], F32, name="sqs")
            zc = constp.tile([128, 1], F32, name="zc")
            nc.vector.memset(zc[:, :], 0.0)
            # preload the exp ACT table while the first DMAs run
            warm = constp.tile([128, 1], F32, name="warm")
            nc.scalar.activation(warm[:, :], zc[:, :], AF.Exp, bias=zc[:, :])

            # --- single SP DMA stream in exact consumption order on the
            # serial DMA engine: wg chunk 0-3 | h0 piece 0 | wg rest |
            # h0 piece 1 | wkv half | h0 piece 2 | wkv half + bias |
            # h0 rest ... then 4kc pieces per group, cswn after group 1,
            # per-group out DMAs at the end ---
            wgb_sb = constp.tile([128, KC * D + GM], BF16, name="wgb_sb")
            bias4_sb = wgb_sb[:, :GM]
            wg_sb = wgb_sb[:, GM:]
            wkv_sb = constp.tile([128, KC * D], BF16, name="wkv_sb")
            cswn_sb = constp.tile([128, 2 * ROPE_DIM + D], F32, name="cswn_sb")
            cs_sb = cswn_sb[:, : 2 * ROPE_DIM]
            wn_sb = cswn_sb[:, 2 * ROPE_DIM :]

            hT_src = hT_in.rearrange("(kc p) t -> p kc t", p=128)
            hts = []
            for g in range(GROUPS):
                hts.append(hTp.tile([128, KC * GM], BF16, name="hT", tag="hT"))

            def h_piece(g, k0, k1):
                htv = hts[g].rearrange("p (kc t) -> p kc t", kc=KC)
                src = hT_src[:, :, g * GM : (g + 1) * GM]
                nc.sync.dma_start(out=htv[:, k0:k1, :], in_=src[:, k0:k1, :])

            def wg_piece(k0, k1):
                nc.sync.dma_start(
                    out=wgb_sb[:, GM + k0 * D : GM + k1 * D],
                    in_=wgb_in[:, GM + k0 * D : GM + k1 * D],
                )

            # [bias | wg kc0-3] ride the very first transfer
            nc.sync.dma_start(
                out=wgb_sb[:, : GM + 4 * D], in_=wgb_in[:, : GM + 4 * D]
            )
            h_piece(0, 0, 2)
            nc.sync.dma_start(out=wkv_sb[:, : 8 * D], in_=wkv_in[:, : 8 * D])
            h_piece(0, 2, 4)
            wg_piece(4, 12)
            h_piece(0, 4, 6)
            h_piece(0, 6, 8)
            wg_piece(12, 16)
            nc.sync.dma_start(out=wkv_sb[:, 8 * D :], in_=wkv_in[:, 8 * D :])
            h_piece(0, 8, 10)
            h_piece(0, 10, 12)
            h_piece(0, 12, 14)
            h_piece(0, 14, 16)
            for g in range(1, GROUPS):
                h_piece(g, 0, 8)
                h_piece(g, 8, 16)
                if g == 1:
                    nc.sync.dma_start(out=cswn_sb, in_=cswn_in[:, :])

            def softmax_front(g, gt_ps, kv_ps):
                # one big exp (ACT), then DVE: denominator reduce + recip
                # (ready at exp-end, overlapping the kv tail), e*kv,
                # numerator reduce, comp = num/den
                ep = esbp.tile([D, 2 * GM], F32, name="ep", tag="ep")
                den4 = smallp.tile([D, GW], F32, name="den4", tag="den")
                rden = smallp.tile([D, GW], F32, name="rden", tag="rden")
                nc.scalar.activation(
                    ep[:, :GM], gt_ps[:, :], AF.Exp, bias=zc[:D, :]
                )
                nc.vector.tensor_reduce(
                    den4[:, :],
                    ep[:, :GM].rearrange("p (w m) -> p w m", w=GW),
                    axis=mybir.AxisListType.X,
                    op=ALU.add,
                )
                nc.vector.reciprocal(rden[:, :], den4[:, :])
                nc.vector.tensor_mul(ep[:, GM:], ep[:, :GM], kv_ps[:, :])
                nd = smallp.tile([D, GW], F32, name="nd", tag="nd")
                nc.vector.tensor_reduce(
                    nd[:, :],
                    ep[:, GM:].rearrange("p (w m) -> p w m", w=GW),
                    axis=mybir.AxisListType.X,
                    op=ALU.add,
                )
                nc.vector.tensor_mul(
                    comp[:, g * GW : (g + 1) * GW], nd[:, :], rden[:, :]
                )

            def backhalf():
                # single fused epilogue: one transpose [128,16] -> [16,128],
                # one RMS+RoPE chain on all 16 window rows (partition-
                # parallel ops cost the same for 4 or 16 rows), ONE out-DMA
                W = WPC
                ct16 = ctp.tile([W, D], F32, name="ct16")
                nc.tensor.transpose(ct16[:, :], comp[:, :], ident[:, :])
                # one ACT Square with fused row-sum replaces copy+mul+reduce
                # (ACT is idle here and reads PSUM directly)
                nc.scalar.activation(
                    sqs[:W, :], ct16[:, :], AF.Square,
                    bias=zc[:W, :], accum_out=ssq[:W, :],
                )
                # rinv = 1/sqrt(ssq/D + eps): magic-constant guess + one
                # Newton step (~0.2% err, well inside tolerance)
                vv = finalp.tile([128, 1], F32, name="vv")
                rinv = finalp.tile([128, 1], F32, name="rinv")
                nt = finalp.tile([128, 1], F32, name="nt")
                vvg = vv[:W, :]
                rig = rinv[:W, :]
                ntg = nt[:W, :]
                nc.vector.tensor_scalar(
                    out=vvg, in0=ssq[:W, :],
                    scalar1=1.0 / D, scalar2=EPS, op0=ALU.mult, op1=ALU.add,
                )
                nc.vector.tensor_scalar(
                    out=rig.bitcast(I32), in0=vvg.bitcast(I32),
                    scalar1=1, scalar2=None, op0=ALU.arith_shift_right,
                )
                nc.vector.tensor_scalar(
                    out=rig.bitcast(I32), in0=rig.bitcast(I32),
                    scalar1=-1, scalar2=None, op0=ALU.bitwise_xor,
                )
                nc.vector.tensor_scalar(
                    out=rig.bitcast(I32), in0=rig.bitcast(I32),
                    scalar1=0x5F3759DF + 1, scalar2=None, op0=ALU.add,
                )
                nc.vector.tensor_mul(ntg, rig, rig)
                nc.vector.tensor_mul(ntg, ntg, vvg)
                nc.vector.tensor_scalar(
                    out=ntg, in0=ntg,
                    scalar1=-0.5, scalar2=1.5, op0=ALU.mult, op1=ALU.add,
                )
                nc.vector.tensor_mul(rig, rig, ntg)

                # wn is folded into the host-side cs tables for the rope
                # half; the nope half gets a [16,64] wn multiply
                og = out_sb[:W, :]
                nc.vector.tensor_scalar_mul(og, ct16[:, :], rig)
                nc.vector.tensor_mul(
                    og[:, : D - ROPE_DIM],
                    og[:, : D - ROPE_DIM],
                    wn_sb[:W, : D - ROPE_DIM],
                )
                # RoPE on the last 64 channels (sign folded into cs table)
                t1 = finalp.tile([128, ROPE_DIM], F32, name="t1")
                t2 = finalp.tile([128, ROPE_DIM], F32, name="t2")
                nc.vector.tensor_mul(
                    t1[:W, :], og[:, D - ROPE_DIM : D], cs_sb[:W, 0:ROPE_DIM]
                )
                nc.vector.tensor_mul(
                    t2[:W, 0:HALF], og[:, D - HALF : D],
                    cs_sb[:W, ROPE_DIM : ROPE_DIM + HALF],
                )
                nc.vector.tensor_mul(
                    t2[:W, HALF:ROPE_DIM], og[:, D - ROPE_DIM : D - HALF],
                    cs_sb[:W, ROPE_DIM + HALF : 2 * ROPE_DIM],
                )
                nc.vector.tensor_add(
                    og[:, D - ROPE_DIM : D], t1[:W, :], t2[:W, :]
                )
                # nope half fires as soon as the wn multiply lands (SP);
                # rope half rides the idle ACT queue -- the descriptor
                # generations overlap instead of serializing
                nc.sync.dma_start(
                    out=out_d[:, : D - ROPE_DIM], in_=og[:, : D - ROPE_DIM]
                )
                nc.scalar.dma_start(
                    out=out_d[:, D - ROPE_DIM :], in_=og[:, D - ROPE_DIM :]
                )

            # all four bias matmuls first: real PE work that only needs
            # the tiny first DMA -- burns the clock-ramp window and the
            # group-0 h-chase stalls
            gts = [
                gtp.tile([D, GM], F32, name="gt_ps", tag="gt")
                for _ in range(GROUPS)
            ]
            for g in range(GROUPS):
                nc.tensor.matmul(
                    gts[g][:, :], ident_bf[:, :], bias4_sb,
                    start=True, stop=False, skip_group_check=True,
                )

            for g in range(GROUPS):
                ht = hts[g]
                gt_ps = gts[g]
                kv_ps = kvp.tile([D, GM], F32, name="kv_ps", tag="kv")

                def gate_q(q):
                    for k in range(4 * q, 4 * q + 4):
                        nc.tensor.matmul(
                            gt_ps[:, :],
                            wg_sb[:, k * D : (k + 1) * D],
                            ht[:, k * GM : (k + 1) * GM],
                            start=False,
                            stop=(k == KC - 1),
                            skip_group_check=True,
                        )

                def kv_q(q):
                    for k in range(4 * q, 4 * q + 4):
                        nc.tensor.matmul(
                            kv_ps[:, :],
                            wkv_sb[:, k * D : (k + 1) * D],
                            ht[:, k * GM : (k + 1) * GM],
                            start=(k == 0),
                            stop=(k == KC - 1),
                            skip_group_check=True,
                        )

                # gate finishes two quartets before the group ends: the exp
                # and the denominator reduce+recip complete inside the kv
                # tail, so only e*kv work remains after the last matmul
                gate_q(0)
                kv_q(0)
                gate_q(1)
                kv_q(1)
                gate_q(2)
                gate_q(3)
                kv_q(2)
                kv_q(3)
                softmax_front(g, gt_ps, kv_ps)
            backhalf()

    nc.compile()
    return nc


_NC_CACHE = {}


def _get_nc():
    if "nc" not in _NC_CACHE:
        _NC_CACHE["nc"] = _build_nc()
    return _NC_CACHE["nc"]


def _make_in_maps(hidden_states, w_kv, w_gate, position_bias, kv_norm_weight):
    hidden_states = np.asarray(hidden_states, dtype=np.float32)
    w_kv = np.asarray(w_kv, dtype=np.float32)
    w_gate = np.asarray(w_gate, dtype=np.float32)
    position_bias = np.asarray(position_bias, dtype=np.float32)
    kv_norm_weight = np.asarray(kv_norm_weight, dtype=np.float32)

    h_flat = hidden_states.reshape(B * S, H)
    # weights to [p, kc, d] bf16 (contiguous per-partition DMA rows)
    wkv_p = np.ascontiguousarray(
        w_kv.reshape(KC, 128, D).transpose(1, 0, 2).reshape(128, KC * D)
    ).astype(NP_BF16)
    wg_p = (
        w_gate.reshape(KC, 128, D).transpose(1, 0, 2).reshape(128, KC * D)
    ).astype(NP_BF16)
    bias4 = np.tile(position_bias.T, (1, GW)).astype(NP_BF16)
    wgb = np.ascontiguousarray(np.concatenate([bias4, wg_p], axis=1))
    wn = np.broadcast_to(kv_norm_weight[None, :], (128, D)).astype(np.float32)

    inv_freq = (1.0 / (THETA ** (np.arange(HALF, dtype=np.float32) / HALF))).astype(
        np.float32
    )
    in_maps = []
    for c in range(NCORES):
        hT = np.ascontiguousarray(
            h_flat[c * WPC * M : (c + 1) * WPC * M].T
        ).astype(NP_BF16)

        t_global = (c % (T // WPC)) * WPC + np.arange(WPC, dtype=np.float32)
        pos = (t_global * M).astype(np.float32)
        freqs = pos[:, None] * inv_freq[None, :]
        cos2 = np.repeat(np.cos(freqs), 2, axis=1).astype(np.float32)
        sin2 = np.repeat(np.sin(freqs), 2, axis=1).astype(np.float32)
        # fold the rms weight for the rope half into the tables (per source
        # channel of each product): t1 <- ch 64+j, t2[:32] <- ch 96+j,
        # t2[32:] <- ch 64+j-32
        kw = kv_norm_weight
        cos2 = cos2 * kw[None, D - ROPE_DIM : D]
        sinf = np.concatenate(
            [
                -sin2[:, :HALF] * kw[None, D - HALF : D],
                sin2[:, HALF:] * kw[None, D - ROPE_DIM : D - HALF],
            ],
            axis=1,
        )
        cs16 = np.concatenate([cos2, sinf], axis=1)  # [16, 128]
        # windows live on partition rows 0..15 on-device
        cs = np.zeros((128, 2 * ROPE_DIM), np.float32)
        cs[:WPC] = cs16
        cswn = np.ascontiguousarray(np.concatenate([cs, wn], axis=1))
        in_maps.append(
            {
                "hT_in": hT,
                "wgb_in": wgb,
                "wkv_in": wkv_p,
                "cswn_in": cswn,
            }
        )
    return in_maps


def _assemble(results):
    full = np.concatenate([r["out_d"] for r in results], axis=0)  # [128, 128]
    return full.reshape(B, 1, T, D).astype(np.float32)


def _run(inputs, trace=False, **spmd_kwargs):
    nc = _get_nc()
    in_maps = _make_in_maps(
        inputs["hidden_states"],
        inputs["w_kv"],
        inputs["w_gate"],
        inputs["position_bias"],
        inputs["kv_norm_weight"],
    )
    res = run_bass_kernel_spmd(
        nc, in_maps, core_ids=list(range(NCORES)), trace=trace, **spmd_kwargs
    )
    return _assemble(res.results), res


def kernel(
    hidden_states,
    q_residual=None,
    position_ids=None,
    w_kv=None,
    w_gate=None,
    position_bias=None,
    kv_norm_weight=None,
):
    out, _ = _run(
        {
            "hidden_states": hidden_states,
            "w_kv": w_kv,
            "w_gate": w_gate,
            "position_bias": position_bias,
            "kv_norm_weight": kv_norm_weight,
        }
    )
    return out


# revision 37
# speedup vs baseline: 1.0397x; 1.0397x over previous
"""Trainium2 Bass kernel for DeepseekV4 HCA compressor (single-shot window compression).

Computation per 128-token window:
    kv   = h @ w_kv            [128, 128]
    gate = h @ w_gate + bias   [128, 128]
    w    = softmax(gate, axis=tokens)   (per output channel)
    comp = sum(w * kv, axis=tokens)     [128]
then RMS-norm over channels and interleaved RoPE on the last 64 channels.

Sharding: 128 windows (2 batches x 64) split across 8 cores, 16 windows each.

Layout strategy: h is uploaded PRE-TRANSPOSED per core ([H, tokens]) in bf16,
so the PE does zero transposes -- just the two bf16 projection matmuls at
full PE rate plus a tiny identity matmul folding in the position bias.

DMA transfers serialize on one shared engine (~360 GB/s/core), so the
whole input rides ONE SP-queue stream in exact consumption order:
[bias|wg chunks interleaved with h group-0 pieces, wkv halves], then 4kc
h pieces per group (cswn mid-stream). The PE is the critical engine
(~29us of bf16 matmuls vs ~27us of wire), so no dummy warmup: the clock
ramp is burned on the four bias matmuls + the group-0 DMA chase.

Per group: bias matmul opens the gate PSUM group (all four emitted first),
gate/kv matmuls interleave per 4-chunk quartet with the gate group closing
two quartets early -- the exp (ACT) and denominator reduce+recip (DVE)
finish inside the kv tail, leaving only e*kv + numerator reduce + num/den
after the last matmul. Epilogue is fused and runs once: one PE transpose
[128,16]->[16,128], ACT Square with fused row-sum straight off PSUM, DVE
1-Newton fast-rsqrt, scale (wn folded into the host cs tables for the
rope half), RoPE, and a two-queue output DMA (nope half on SP, rope half
on ACT) so the descriptor generations overlap.
"""

import sys

if "/opt/trn_rl_repo" not in sys.path:
    sys.path.insert(0, "/opt/trn_rl_repo")

import ml_dtypes
import numpy as np

import concourse.bacc as bacc
import concourse.mybir as mybir
import concourse.tile as tile
from concourse.bass_utils import run_bass_kernel_spmd
from concourse.masks import make_identity

# Problem shapes (hardcoded per contest contract)
B, S, H = 2, 8192, 2048
M = 128          # compress rate (window length)
D = 128          # head dim
T = S // M       # 64 windows per batch
NCORES = 8
WPC = (B * T) // NCORES   # 16 windows per core
GW = 4                    # windows per group (-> moving dim 512)
GROUPS = WPC // GW        # 4
KC = H // 128             # 16 contraction chunks
GM = GW * M               # 512 moving tokens per group
ROPE_DIM = 64
HALF = ROPE_DIM // 2
THETA = 10000.0
EPS = 1e-6

F32 = mybir.dt.float32
BF16 = mybir.dt.bfloat16
I32 = mybir.dt.int32
AF = mybir.ActivationFunctionType
ALU = mybir.AluOpType

NP_BF16 = ml_dtypes.bfloat16

# No PE warmup: the PE is the critical engine (slower per h-chunk than the
# DMA stream), so burning PE time on dummies to buy clock ramp is a net
# loss -- the ramp period overlaps the group-0 DMA chase instead.


def _build_nc():
    nc = bacc.Bacc(None, target_bir_lowering=False)

    # h for this core, transposed on host: hT[h, t] = h[t, h], bf16
    hT_in = nc.dram_tensor("hT_in", [H, WPC * M], BF16, kind="ExternalInput")
    # w_gate pre-permuted to [p, kc, d] plus the 4x-tiled position bias
    wgb_in = nc.dram_tensor("wgb_in", [128, KC * D + GM], BF16, kind="ExternalInput")
    wkv_in = nc.dram_tensor("wkv_in", [128, KC * D], BF16, kind="ExternalInput")
    # cos/sin table (128 cols) + rms weight row-broadcast (128 cols)
    cswn_in = nc.dram_tensor("cswn_in", [128, 2 * ROPE_DIM + D], F32, kind="ExternalInput")
    out_d = nc.dram_tensor("out_d", [WPC, D], F32, kind="ExternalOutput")

    with tile.TileContext(nc) as tc:
        with (
            tc.tile_pool(name="constp", bufs=1) as constp,
            tc.tile_pool(name="hTp", bufs=4) as hTp,
            tc.tile_pool(name="esbp", bufs=2) as esbp,
            tc.tile_pool(name="smallp", bufs=2) as smallp,
            tc.tile_pool(name="gtp", bufs=4, space="PSUM") as gtp,
            tc.tile_pool(name="kvp", bufs=3, space="PSUM") as kvp,
            tc.tile_pool(name="ctp", bufs=1, space="PSUM") as ctp,
            tc.tile_pool(name="finalp", bufs=1) as finalp,
        ):
            # --- constants / epilogue state ---
            ident = constp.tile([128, 128], F32, name="ident")
            make_identity(nc, ident)
            ident_bf = constp.tile([128, 128], BF16, name="ident_bf")
            nc.vector.tensor_copy(ident_bf[:, :], ident[:, :])

            comp = constp.tile([D, WPC], F32, name="comp")
            out_sb = finalp.tile([128, D], F32, name="out_sb")
            ssq = finalp.tile([128, 1], F32, name="ssq")
            sqs = finalp.tile([128, D], F32, name="sqs")
            zc = constp.tile([128, 1], F32, name="zc")
            nc.vector.memset(zc[:, :], 0.0)
            # preload the exp ACT table while the first DMAs run
            warm = constp.tile([128, 1], F32, name="warm")
            nc.scalar.activation(warm[:, :], zc[:, :], AF.Exp, bias=zc[:, :])

            # --- single SP DMA stream in exact consumption order on the
            # serial DMA engine: wg chunk 0-3 | h0 piece 0 | wg rest |
            # h0 piece 1 | wkv half | h0 piece 2 | wkv half + bias |
            # h0 rest ... then 4kc pieces per group, cswn after group 1,
            # per-group out DMAs at the end ---
            wgb_sb = constp.tile([128, KC * D + GM], BF16, name="wgb_sb")
            bias4_sb = wgb_sb[:, :GM]
            wg_sb = wgb_sb[:, GM:]
            wkv_sb = constp.tile([128, KC * D], BF16, name="wkv_sb")
            cswn_sb = constp.tile([128, 2 * ROPE_DIM + D], F32, name="cswn_sb")
            cs_sb = cswn_sb[:, : 2 * ROPE_DIM]
            wn_sb = cswn_sb[:, 2 * ROPE_DIM :]

            hT_src = hT_in.rearrange("(kc p) t -> p kc t", p=128)
            hts = []
            for g in range(GROUPS):
                hts.append(hTp.tile([128, KC * GM], BF16, name="hT", tag="hT"))

            def h_piece(g, k0, k1):
                htv = hts[g].rearrange("p (kc t) -> p kc t", kc=KC)
                src = hT_src[:, :, g * GM : (g + 1) * GM]
                nc.sync.dma_start(out=htv[:, k0:k1, :], in_=src[:, k0:k1, :])

            def wg_piece(k0, k1):
                nc.sync.dma_start(
                    out=wgb_sb[:, GM + k0 * D : GM + k1 * D],
                    in_=wgb_in[:, GM + k0 * D : GM + k1 * D],
                )

            # [bias | wg kc0-3] ride the very first transfer
            nc.sync.dma_start(
                out=wgb_sb[:, : GM + 4 * D], in_=wgb_in[:, : GM + 4 * D]
            )
            h_piece(0, 0, 2)
            nc.sync.dma_start(out=wkv_sb[:, : 8 * D], in_=wkv_in[:, : 8 * D])
            h_piece(0, 2, 4)
            wg_piece(4, 12)
            h_piece(0, 4, 6)
            h_piece(0, 6, 8)
            wg_piece(12, 16)
            nc.sync.dma_start(out=wkv_sb[:, 8 * D :], in_=wkv_in[:, 8 * D :])
            h_piece(0, 8, 10)
            h_piece(0, 10, 12)
            h_piece(0, 12, 14)
            h_piece(0, 14, 16)
            for g in range(1, GROUPS):
                for q in range(8):
                    h_piece(g, 2 * q, 2 * q + 2)
                if g == 1:
                    nc.sync.dma_start(out=cswn_sb, in_=cswn_in[:, :])

            def softmax_front(g, gt_ps, kv_ps):
                # one big exp (ACT), then DVE: denominator reduce + recip
                # (ready at exp-end, overlapping the kv tail), e*kv,
                # numerator reduce, comp = num/den
                ep = esbp.tile([D, 2 * GM], F32, name="ep", tag="ep")
                den4 = smallp.tile([D, GW], F32, name="den4", tag="den")
                rden = smallp.tile([D, GW], F32, name="rden", tag="rden")
                nc.scalar.activation(
                    ep[:, :GM], gt_ps[:, :], AF.Exp, bias=zc[:D, :]
                )
                nc.vector.tensor_reduce(
                    den4[:, :],
                    ep[:, :GM].rearrange("p (w m) -> p w m", w=GW),
                    axis=mybir.AxisListType.X,
                    op=ALU.add,
                )
                nc.vector.reciprocal(rden[:, :], den4[:, :])
                nc.vector.tensor_mul(ep[:, GM:], ep[:, :GM], kv_ps[:, :])
                nd = smallp.tile([D, GW], F32, name="nd", tag="nd")
                nc.vector.tensor_reduce(
                    nd[:, :],
                    ep[:, GM:].rearrange("p (w m) -> p w m", w=GW),
                    axis=mybir.AxisListType.X,
                    op=ALU.add,
                )
                nc.vector.tensor_mul(
                    comp[:, g * GW : (g + 1) * GW], nd[:, :], rden[:, :]
                )

            def backhalf():
                # single fused epilogue: one transpose [128,16] -> [16,128],
                # one RMS+RoPE chain on all 16 window rows (partition-
                # parallel ops cost the same for 4 or 16 rows), ONE out-DMA
                W = WPC
                ct16 = ctp.tile([W, D], F32, name="ct16")
                nc.tensor.transpose(ct16[:, :], comp[:, :], ident[:, :])
                # one ACT Square with fused row-sum replaces copy+mul+reduce
                # (ACT is idle here and reads PSUM directly)
                nc.scalar.activation(
                    sqs[:W, :], ct16[:, :], AF.Square,
                    bias=zc[:W, :], accum_out=ssq[:W, :],
                )
                # rinv = 1/sqrt(ssq/D + eps): magic-constant guess + one
                # Newton step (~0.2% err, well inside tolerance)
                vv = finalp.tile([128, 1], F32, name="vv")
                rinv = finalp.tile([128, 1], F32, name="rinv")
                nt = finalp.tile([128, 1], F32, name="nt")
                vvg = vv[:W, :]
                rig = rinv[:W, :]
                ntg = nt[:W, :]
                nc.vector.tensor_scalar(
                    out=vvg, in0=ssq[:W, :],
                    scalar1=1.0 / D, scalar2=EPS, op0=ALU.mult, op1=ALU.add,
                )
                nc.vector.tensor_scalar(
                    out=rig.bitcast(I32), in0=vvg.bitcast(I32),
                    scalar1=1, scalar2=None, op0=ALU.arith_shift_right,
                )
                nc.vector.tensor_scalar(
                    out=rig.bitcast(I32), in0=rig.bitcast(I32),
                    scalar1=-1, scalar2=None, op0=ALU.bitwise_xor,
                )
                nc.vector.tensor_scalar(
                    out=rig.bitcast(I32), in0=rig.bitcast(I32),
                    scalar1=0x5F3759DF + 1, scalar2=None, op0=ALU.add,
                )
                nc.vector.tensor_mul(ntg, rig, rig)
                nc.vector.tensor_mul(ntg, ntg, vvg)
                nc.vector.tensor_scalar(
                    out=ntg, in0=ntg,
                    scalar1=-0.5, scalar2=1.5, op0=ALU.mult, op1=ALU.add,
                )
                nc.vector.tensor_mul(rig, rig, ntg)

                # wn is folded into the host-side cs tables for the rope
                # half; the nope half gets a [16,64] wn multiply
                og = out_sb[:W, :]
                nc.vector.tensor_scalar_mul(og, ct16[:, :], rig)
                nc.vector.tensor_mul(
                    og[:, : D - ROPE_DIM],
                    og[:, : D - ROPE_DIM],
                    wn_sb[:W, : D - ROPE_DIM],
                )
                # RoPE on the last 64 channels (sign folded into cs table)
                t1 = finalp.tile([128, ROPE_DIM], F32, name="t1")
                t2 = finalp.tile([128, ROPE_DIM], F32, name="t2")
                nc.vector.tensor_mul(
                    t1[:W, :], og[:, D - ROPE_DIM : D], cs_sb[:W, 0:ROPE_DIM]
                )
                nc.vector.tensor_mul(
                    t2[:W, 0:HALF], og[:, D - HALF : D],
                    cs_sb[:W, ROPE_DIM : ROPE_DIM + HALF],
                )
                nc.vector.tensor_mul(
                    t2[:W, HALF:ROPE_DIM], og[:, D - ROPE_DIM : D - HALF],
                    cs_sb[:W, ROPE_DIM + HALF : 2 * ROPE_DIM],
                )
                nc.vector.tensor_add(
                    og[:, D - ROPE_DIM : D], t1[:W, :], t2[:W, :]
                )
                # nope half fires as soon as the wn multiply lands (SP);
                # rope half rides the idle ACT queue -- the descriptor
                # generations overlap instead of serializing
                nc.sync.dma_start(
                    out=out_d[:, : D - ROPE_DIM], in_=og[:, : D - ROPE_DIM]
                )
                nc.scalar.dma_start(
                    out=out_d[:, D - ROPE_DIM :], in_=og[:, D - ROPE_DIM :]
                )

            # all four bias matmuls first: real PE work that only needs
            # the tiny first DMA -- burns the clock-ramp window and the
            # group-0 h-chase stalls
            gts = [
                gtp.tile([D, GM], F32, name="gt_ps", tag="gt")
                for _ in range(GROUPS)
            ]
            for g in range(GROUPS):
                nc.tensor.matmul(
                    gts[g][:, :], ident_bf[:, :], bias4_sb,
                    start=True, stop=False, skip_group_check=True,
                )

            for g in range(GROUPS):
                ht = hts[g]
                gt_ps = gts[g]
                kv_ps = kvp.tile([D, GM], F32, name="kv_ps", tag="kv")

                def gate_q(q):
                    for k in range(4 * q, 4 * q + 4):
                        nc.tensor.matmul(
                            gt_ps[:, :],
                            wg_sb[:, k * D : (k + 1) * D],
                            ht[:, k * GM : (k + 1) * GM],
                            start=False,
                            stop=(k == KC - 1),
                            skip_group_check=True,
                        )

                def kv_q(q):
                    for k in range(4 * q, 4 * q + 4):
                        nc.tensor.matmul(
                            kv_ps[:, :],
                            wkv_sb[:, k * D : (k + 1) * D],
                            ht[:, k * GM : (k + 1) * GM],
                            start=(k == 0),
                            stop=(k == KC - 1),
                            skip_group_check=True,
                        )

                # gate finishes two quartets before the group ends: the exp
                # and the denominator reduce+recip complete inside the kv
                # tail, so only e*kv work remains after the last matmul
                gate_q(0)
                kv_q(0)
                gate_q(1)
                kv_q(1)
                gate_q(2)
                gate_q(3)
                kv_q(2)
                kv_q(3)
                softmax_front(g, gt_ps, kv_ps)
            backhalf()

    nc.compile()
    return nc


_NC_CACHE = {}


def _get_nc():
    if "nc" not in _NC_CACHE:
        _NC_CACHE["nc"] = _build_nc()
    return _NC_CACHE["nc"]


def _make_in_maps(hidden_states, w_kv, w_gate, position_bias, kv_norm_weight):
    hidden_states = np.asarray(hidden_states, dtype=np.float32)
    w_kv = np.asarray(w_kv, dtype=np.float32)
    w_gate = np.asarray(w_gate, dtype=np.float32)
    position_bias = np.asarray(position_bias, dtype=np.float32)
    kv_norm_weight = np.asarray(kv_norm_weight, dtype=np.float32)

    h_flat = hidden_states.reshape(B * S, H)
    # weights to [p, kc, d] bf16 (contiguous per-partition DMA rows)
    wkv_p = np.ascontiguousarray(
        w_kv.reshape(KC, 128, D).transpose(1, 0, 2).reshape(128, KC * D)
    ).astype(NP_BF16)
    wg_p = (
        w_gate.reshape(KC, 128, D).transpose(1, 0, 2).reshape(128, KC * D)
    ).astype(NP_BF16)
    bias4 = np.tile(position_bias.T, (1, GW)).astype(NP_BF16)
    wgb = np.ascontiguousarray(np.concatenate([bias4, wg_p], axis=1))
    wn = np.broadcast_to(kv_norm_weight[None, :], (128, D)).astype(np.float32)

    inv_freq = (1.0 / (THETA ** (np.arange(HALF, dtype=np.float32) / HALF))).astype(
        np.float32
    )
    in_maps = []
    for c in range(NCORES):
        hT = np.ascontiguousarray(
            h_flat[c * WPC * M : (c + 1) * WPC * M].T
        ).astype(NP_BF16)

        t_global = (c % (T // WPC)) * WPC + np.arange(WPC, dtype=np.float32)
        pos = (t_global * M).astype(np.float32)
        freqs = pos[:, None] * inv_freq[None, :]
        cos2 = np.repeat(np.cos(freqs), 2, axis=1).astype(np.float32)
        sin2 = np.repeat(np.sin(freqs), 2, axis=1).astype(np.float32)
        # fold the rms weight for the rope half into the tables (per source
        # channel of each product): t1 <- ch 64+j, t2[:32] <- ch 96+j,
        # t2[32:] <- ch 64+j-32
        kw = kv_norm_weight
        cos2 = cos2 * kw[None, D - ROPE_DIM : D]
        sinf = np.concatenate(
            [
                -sin2[:, :HALF] * kw[None, D - HALF : D],
                sin2[:, HALF:] * kw[None, D - ROPE_DIM : D - HALF],
            ],
            axis=1,
        )
        cs16 = np.concatenate([cos2, sinf], axis=1)  # [16, 128]
        # windows live on partition rows 0..15 on-device
        cs = np.zeros((128, 2 * ROPE_DIM), np.float32)
        cs[:WPC] = cs16
        cswn = np.ascontiguousarray(np.concatenate([cs, wn], axis=1))
        in_maps.append(
            {
                "hT_in": hT,
                "wgb_in": wgb,
                "wkv_in": wkv_p,
                "cswn_in": cswn,
            }
        )
    return in_maps


def _assemble(results):
    full = np.concatenate([r["out_d"] for r in results], axis=0)  # [128, 128]
    return full.reshape(B, 1, T, D).astype(np.float32)


def _run(inputs, trace=False, **spmd_kwargs):
    nc = _get_nc()
    in_maps = _make_in_maps(
        inputs["hidden_states"],
        inputs["w_kv"],
        inputs["w_gate"],
        inputs["position_bias"],
        inputs["kv_norm_weight"],
    )
    res = run_bass_kernel_spmd(
        nc, in_maps, core_ids=list(range(NCORES)), trace=trace, **spmd_kwargs
    )
    return _assemble(res.results), res


def kernel(
    hidden_states,
    q_residual=None,
    position_ids=None,
    w_kv=None,
    w_gate=None,
    position_bias=None,
    kv_norm_weight=None,
):
    out, _ = _run(
        {
            "hidden_states": hidden_states,
            "w_kv": w_kv,
            "w_gate": w_gate,
            "position_bias": position_bias,
            "kv_norm_weight": kv_norm_weight,
        }
    )
    return out
